# revision 10
# baseline (speedup 1.0000x reference)
"""AURC loss kernel for Trainium2, sharded across 8 NeuronCores.

Algorithm (matches the reference):
  logp = log_softmax(x);  score = exp(max logp);  loss = -logp[target]
  sort by score desc;  result = sum_i cumsum(sorted_loss)[i-1]/i / B
Rewritten rank-wise: result = sum_j loss_j * w[rank_j] where
  rank_j = #{m : key_m > key_j},  key = max(x) - logsumexp(x)  (monotone in score)
  w[r] = (H_{B-1} - H_r) / B,  H_r = sum_{i=1}^r 1/i   (precomputed table)

Sharding: batch B=8192 split 1024 rows/core. Each core streams its
[1024, 32000] shard once (row max via DVE, exp-sum via the ACT accumulator,
target logit via indirect DMA), AllGathers the 8192 key scalars, counts
ranks for its own 1024 keys against the gathered 8192, gathers w[rank]
from an inline table, and emits a partial dot product with its local
losses. Host sums the 8 partials.
"""
import sys

if "/opt/trn_rl_repo" not in sys.path:
    sys.path.insert(0, "/opt/trn_rl_repo")

import numpy as np

B, C = 8192, 32000
NCORES = 8
BL = B // NCORES          # rows per core
P = 128                   # partitions
NG = BL // P              # row-groups per core
CF = 8000                 # columns per streamed chunk
NCH = C // CF             # chunks per row-group

_CACHE = {}


def _w_table() -> np.ndarray:
    h = np.zeros(B, dtype=np.float64)
    h[1:] = np.cumsum(1.0 / np.arange(1, B, dtype=np.float64))
    return ((h[B - 1] - h) / B).astype(np.float32)


def _build():
    import concourse.bass as bass
    import concourse.bacc as bacc
    import concourse.mybir as mybir
    import concourse.tile as tile

    nc = bacc.Bacc(num_devices=NCORES)
    x = nc.dram_tensor("x", [BL, C], mybir.dt.float32, kind="ExternalInput")
    # flat element offsets r*C + tgt[r], laid out so tile[p, g] = row g*P+p
    toff = nc.dram_tensor("toff", [BL], mybir.dt.int32, kind="ExternalInput")
    out = nc.dram_tensor("out", [1, 1], mybir.dt.float32, kind="ExternalOutput")
    wtab = nc.inline_tensor(_w_table(), name="wtab")

    xflat = x.rearrange("a b -> (a b)").unsqueeze(1)
    f32 = mybir.dt.float32
    i32 = mybir.dt.int32
    AX = mybir.AxisListType.X
    OP = mybir.AluOpType
    AF = mybir.ActivationFunctionType

    with tile.TileContext(nc) as tc:
        with (
            tc.tile_pool(name="sb", bufs=3) as sb,
            tc.tile_pool(name="sm", bufs=1) as sm,
            tc.tile_pool(name="ps", bufs=1, space="PSUM") as ps,
            tc.tile_pool(name="dr", bufs=1, space="DRAM") as dr,
        ):
            keys = sm.tile([P, NG], f32)
            losses = sm.tile([P, NG], f32)
            ones_col = sm.tile([P, 1], f32)
            nc.vector.memset(ones_col[:, :], 1.0)

            # gather target logits x[r, tgt[r]]
            off_t = sm.tile([P, NG], i32)
            nc.sync.dma_start(off_t[:, :], toff.rearrange("(p g) -> p g", g=NG))
            xt = sm.tile([P, NG], f32)
            nc.gpsimd.indirect_dma_start(
                out=xt[:, :], out_offset=None, in_=xflat,
                in_offset=bass.IndirectOffsetOnAxis(ap=off_t[:, :], axis=0))

            # ---- phase 1: stream shard; per-row max and exp-sum ----
            for g in range(NG):
                mx = sm.tile([P, NCH], f32, tag="mx", bufs=2)
                sms = sm.tile([P, NCH], f32, tag="sms", bufs=2)
                for c in range(NCH):
                    t = sb.tile([P, CF], f32, tag="t")
                    nc.sync.dma_start(
                        t[:, :], x[g * P:(g + 1) * P, c * CF:(c + 1) * CF])
                    nc.vector.reduce_max(mx[:, c:c + 1], t[:, :], axis=AX)
                    nc.scalar.activation(out=t[:, :], in_=t[:, :], func=AF.Exp,
                                         accum_out=sms[:, c:c + 1])
                m = sm.tile([P, 1], f32, tag="m", bufs=2)
                s = sm.tile([P, 1], f32, tag="s", bufs=2)
                nc.vector.reduce_max(m[:, :], mx[:, :], axis=AX)
                nc.vector.reduce_sum(s[:, :], sms[:, :], axis=AX)
                lse = sm.tile([P, 1], f32, tag="lse", bufs=2)
                nc.scalar.activation(out=lse[:, :], in_=s[:, :], func=AF.Ln)
                # key = m - lse ; loss = lse - x[target]   (both on ACT:
                # out = in*scale + bias, so each op inline-waits on one dep)
                nc.scalar.activation(out=keys[:, g:g + 1], in_=lse[:, :],
                                     func=AF.Identity, scale=-1.0, bias=m[:, :])
                nc.scalar.activation(out=losses[:, g:g + 1], in_=xt[:, g:g + 1],
                                     func=AF.Identity, scale=-1.0, bias=lse[:, :])

            # ---- phase 2: AllGather the key scalars ----
            kl_in = dr.tile([BL], f32)
            nc.sync.dma_start(kl_in[:].rearrange("(p g) -> p g", g=NG), keys[:, :])
            kl_all = dr.tile([B], f32)
            nc.gpsimd.collective_compute(
                "AllGather", OP.bypass,
                replica_groups=[list(range(NCORES))],
                ins=[kl_in.opt()], outs=[kl_all.opt()])

            # ---- phase 3: ranks and weighted partial sum ----
            # replicate global keys to all partitions via broadcast-read DMA
            grep = sm.tile([P, B], f32)
            nc.sync.dma_start(grep[:, :],
                              kl_all[:].unsqueeze(0).to_broadcast([P, B]))

            ranks = sm.tile([P, NG], f32)
            trash = sm.tile([P, B], f32)
            for g in range(NG):
                nc.vector.tensor_scalar(
                    out=trash[:, :], in0=grep[:, :], scalar1=keys[:, g:g + 1],
                    scalar2=None, op0=OP.is_gt, op1=OP.add,
                    accum_out=ranks[:, g:g + 1])

            rank_i = sm.tile([P, NG], i32)
            nc.vector.tensor_copy(rank_i[:, :], ranks[:, :])
            wg = sm.tile([P, NG], f32)
            nc.gpsimd.indirect_dma_start(
                out=wg[:, :], out_offset=None, in_=wtab[:].unsqueeze(1),
                in_offset=bass.IndirectOffsetOnAxis(ap=rank_i[:, :], axis=0))
            prod = sm.tile([P, NG], f32)
            for g in range(NG):
                nc.scalar.activation(out=prod[:, g:g + 1], in_=wg[:, g:g + 1],
                                     func=AF.Copy, scale=losses[:, g:g + 1])
            prow = sm.tile([P, 1], f32)
            nc.vector.reduce_sum(prow[:, :], prod[:, :], axis=AX)
            pscal = ps.tile([1, 1], f32, tag="pscal")
            nc.tensor.matmul(pscal[:, :], lhsT=prow[:, :], rhs=ones_col[:, :],
                             start=True, stop=True)
            psb = sm.tile([1, 1], f32)
            nc.scalar.copy(psb[:, :], pscal[:, :])
            nc.sync.dma_start(out[:, :], psb[:, :])

    nc.finalize()
    return nc


def _shard_inputs(input: np.ndarray, target: np.ndarray):
    xin = np.ascontiguousarray(input, dtype=np.float32)
    toff = (np.arange(B, dtype=np.int64) % BL) * C + target.astype(np.int64)
    toff = toff.astype(np.int32).reshape(NCORES, NG, P)
    # tile[p, g] = row g*P+p  ->  flat host order (p, g)
    toff = np.ascontiguousarray(toff.transpose(0, 2, 1)).reshape(NCORES, BL)
    return [
        {"x": xin[c * BL:(c + 1) * BL], "toff": toff[c]}
        for c in range(NCORES)
    ]


def _run(input: np.ndarray, target: np.ndarray, trace: bool = False):
    from concourse.bass_utils import run_bass_kernel_spmd

    if "nc" not in _CACHE:
        _CACHE["nc"] = _build()
    nc = _CACHE["nc"]

    in_maps = _shard_inputs(input, target)
    res = run_bass_kernel_spmd(nc, in_maps, core_ids=list(range(NCORES)),
                               trace=trace)
    parts = [r["out"][0, 0] for r in res.results]
    total = np.float32(np.sum(np.asarray(parts, dtype=np.float64)))
    return np.asarray(total, dtype=np.float32), res


def kernel(input: np.ndarray, target: np.ndarray) -> np.ndarray:
    out, _ = _run(input, target, trace=False)
    return out
